# revision 1
# baseline (speedup 1.0000x reference)
"""Trainium2 Bass kernel for nn_Graph_Generator (gnn_message_passing).

Computation (reference):
    E_d    = tanh(einsum('bcnt,cm->bnm', x, E_s))          # [B, N, M]
    scores = relu(einsum('bnm,bkm->bnk', E_d, E_d) / sqrt(C))
    A_adp  = softmax(scores, axis=-1)                      # [B, N, N]
    out    = (A_adp.mean(axis=0) > 0.5).float32            # [N, N]

Strategy: data-parallel over batch B=128 across 8 cores (16 batches/core).
Each core returns its partial sum of softmax outputs [N, N]; the host adds
the 8 partials, divides by B and thresholds.

Per-batch on-device pipeline:
  DMA x[b] -> DVE sum over T -> PE (E_s^T @ xs, 2 row-chunks since N=170>128)
  -> ACT tanh -> PE (E_d E_d^T, PSUM-accumulated over the 170-dim contraction)
  -> ACT relu(x/sqrt(C)) -> ACT exp with free row-sum accum_out
  -> DVE reciprocal -> DVE fused (exp * recip) + acc.
"""

import math
import sys

for _p in ("/opt/trn_rl_repo",):
    if _p not in sys.path:
        sys.path.insert(0, _p)

import numpy as np

import concourse.bacc as bacc
import concourse.bass as bass
import concourse.mybir as mybir
from concourse.tile import TileContext, add_dep_helper
from concourse.bass_utils import run_bass_kernel_spmd

B, C, N, T = 128, 128, 170, 12
NCORES = 8
BLOC = B // NCORES  # batches per core
NA = 128            # first row-chunk of the N dimension
NB = N - NA         # second row-chunk (42)
F32 = mybir.dt.float32
BF16 = mybir.dt.bfloat16
AFT = mybir.ActivationFunctionType
ALU = mybir.AluOpType


def _build_kernel():
    nc = bacc.Bacc(None, target_bir_lowering=False)
    x_in = nc.declare_dram_parameter("x", [BLOC, C, N * T], F32, isOutput=False)
    es_in = nc.declare_dram_parameter("E_s", [C, N], F32, isOutput=False)
    out = nc.declare_dram_parameter("acc", [N, N], F32, isOutput=True)

    scale = 1.0 / math.sqrt(float(C))

    with TileContext(nc) as tc:
        with (
            tc.tile_pool(name="singles", bufs=1) as singles,
            tc.tile_pool(name="xload", bufs=5) as xload,
            tc.tile_pool(name="work", bufs=3) as work,
            tc.tile_pool(name="pp", bufs=2, space="PSUM") as pp,
        ):
            # First x load goes out on the sync HWDGE ring before anything
            # else; E_s rides the gpsimd (SWDGE) ring so it doesn't delay it.
            # Batches 0..13 load as 2-batch pairs (bigger DMAs run closer to
            # peak HBM bandwidth); 14 as a single; 15 as two n-halves.
            F = N * T

            def load_pair(i):
                # tile[c, b, f] <- x_in[i + b, c, f]
                pt = xload.tile([C, 2, F], F32, tag="x")
                nc.sync.dma_start(out=pt.rearrange("c b f -> b c f"),
                                  in_=x_in[i:i + 2])
                return pt

            def load_single(i):
                st = xload.tile([C, F], F32, tag="xsingle")
                nc.sync.dma_start(out=st, in_=x_in[i])
                return st

            # batch -> ('pair', start) | ('single',) ; batch 15 is n-halved.
            PAIR_FIRST, PAIR_LAST = 1, 0  # pairs (1,2)..(9,10)
            single_tiles = {0: load_single(0)}
            pair_tiles = {}

            es_t = singles.tile([C, N], F32)
            nc.gpsimd.dma_start(out=es_t, in_=es_in[:, :])

            acc_a = singles.tile([128, N], F32)
            acc_b = singles.tile([128, N], F32)
            nc.vector.memset(acc_a, 0.0)
            nc.vector.memset(acc_b[:NB], 0.0)

            def make_tail(i, exp_a, exp_b):
                def emit(next_reduce):
                    s2 = work.tile([128, 2], F32, tag="s2")
                    ts_a = nc.vector.tensor_scalar(
                        out=exp_a, in0=exp_a, scalar1=1.0, scalar2=0.0,
                        op0=ALU.max, op1=ALU.add, accum_out=s2[:, 0:1])
                    if next_reduce is not None:
                        add_dep_helper(
                            ts_a.ins, next_reduce.ins, sync=False,
                            reason="pipeline: next batch's reduce first")
                    nc.vector.tensor_scalar(
                        out=exp_b[:NB], in0=exp_b[:NB], scalar1=1.0,
                        scalar2=0.0, op0=ALU.max, op1=ALU.add,
                        accum_out=s2[:NB, 1:2])
                    r2 = work.tile([128, 2], F32, tag="r2")
                    nc.vector.reciprocal(r2, s2)
                    # acc += exp * (1/rowsum).  On the last batch update
                    # acc_b first so its (smaller) store issues while acc_a's
                    # final update still runs.
                    upd_a = (acc_a, exp_a, r2[:, 0:1], slice(0, 128))
                    upd_b = (acc_b, exp_b, r2[:, 1:2], slice(0, NB))
                    for acc_t, exp_t, r_t, rows in (
                            (upd_b, upd_a) if i == BLOC - 1 else (upd_a, upd_b)):
                        nc.vector.scalar_tensor_tensor(
                            out=acc_t[rows], in0=exp_t[rows], scalar=r_t[rows],
                            in1=acc_t[rows], op0=ALU.mult, op1=ALU.add)
                return emit

            pending_tail = None

            for i in range(BLOC):
                pe_a = pp.tile([128, N], F32, tag="pe_a")
                pe_b = pp.tile([128, N], F32, tag="pe_b")
                # tanh output in bf16: matmul2 then runs at 1 cyc/col instead
                # of 4 (and FWL-fast weight loads).  Margin check: A_mean's
                # closest approach to the 0.5 threshold is ~8e-3; bf16 E_d
                # perturbs A_mean by <2e-5.
                ed_a = work.tile([128, N], BF16, tag="ed_a")
                ed_b = work.tile([128, N], BF16, tag="ed_b")

                if i < BLOC - 1:
                    if PAIR_FIRST <= i <= PAIR_LAST:
                        j = i - PAIR_FIRST
                        if j % 2 == 0:
                            pair_tiles[i] = load_pair(i)
                            x_t = pair_tiles[i][:, 0, :]
                        else:
                            x_t = pair_tiles[i - 1][:, 1, :]
                    else:
                        if i not in single_tiles:
                            single_tiles[i] = load_single(i)
                        x_t = single_tiles[i]

                    # xs[c, n] = sum_t x[b, c, n, t].  DVE tensor_reduce only
                    # has a 1x-mode uop (~2.2us per batch), so split the work:
                    # GpSimd folds the two T-halves (T=12 -> 6), DVE reduces
                    # the rest.  Keeps both engines under the ~2.9us/batch DMA
                    # cadence.
                    xs_t = work.tile([C, N], F32, tag="xs")
                    x3 = x_t.rearrange("c (n t) -> c n t", t=T)
                    h1 = work.tile([C, N, 6], F32, tag="h1")
                    nc.gpsimd.tensor_tensor(
                        out=h1, in0=x3[:, :, 0:6], in1=x3[:, :, 6:12], op=ALU.add)
                    red_inst = nc.vector.reduce_sum(
                        xs_t, h1, axis=mybir.AxisListType.X)

                    # E_dT[m, n] = tanh(sum_c E_s[c, m] xs[c, n]); m = 128+42
                    nc.tensor.matmul(pe_a, lhsT=es_t[:, 0:NA], rhs=xs_t,
                                     start=True, stop=True)
                    nc.tensor.matmul(pe_b[:NB], lhsT=es_t[:, NA:N], rhs=xs_t,
                                     start=True, stop=True)
                    nc.scalar.activation(ed_a, pe_a, AFT.Tanh)
                    nc.scalar.activation(ed_b[:NB], pe_b[:NB], AFT.Tanh)
                else:
                    # Last batch: split into two n-halves so its T-sum /
                    # matmul1 / tanh overlap its own load -- this chain is
                    # fully exposed at the end of the DMA stream.
                    NH = N // 2  # 85
                    for j in range(2):
                        xh = xload.tile([C, NH * T], F32, tag="xh")
                        nc.sync.dma_start(
                            out=xh, in_=x_in[i][:, j * NH * T:(j + 1) * NH * T])
                        xh3 = xh.rearrange("c (n t) -> c n t", t=T)
                        h1h = work.tile([C, NH, 6], F32, tag="h1h")
                        nc.gpsimd.tensor_tensor(
                            out=h1h, in0=xh3[:, :, 0:6], in1=xh3[:, :, 6:12],
                            op=ALU.add)
                        xsh = work.tile([C, NH], F32, tag="xsh")
                        r = nc.vector.reduce_sum(
                            xsh, h1h, axis=mybir.AxisListType.X)
                        if j == 0:
                            red_inst = r
                        cols = slice(j * NH, (j + 1) * NH)
                        nc.tensor.matmul(pe_a[:, cols], lhsT=es_t[:, 0:NA],
                                         rhs=xsh, start=True, stop=True)
                        nc.tensor.matmul(pe_b[:NB, cols], lhsT=es_t[:, NA:N],
                                         rhs=xsh, start=True, stop=True)
                        nc.scalar.activation(ed_a[:, cols], pe_a[:, cols],
                                             AFT.Tanh)
                        nc.scalar.activation(ed_b[:NB, cols], pe_b[:NB, cols],
                                             AFT.Tanh)

                # scores[n, k] = sum_m E_dT[m, n] * E_dT[m, k]; n split 128+42,
                # contraction over m accumulated across the two m-chunks.
                # On the last batch run the b-chunk chain first so its store
                # can issue while the a-chunk finishes.
                ps_a = pp.tile([128, N], F32, tag="ps_a")
                ps_b = pp.tile([128, N], F32, tag="ps_b")
                exp_a = work.tile([128, N], F32, tag="exp_a")
                exp_b = work.tile([128, N], F32, tag="exp_b")

                def emit_mm2_a():
                    nc.tensor.matmul(ps_a, lhsT=ed_a[:, 0:NA], rhs=ed_a,
                                     start=True, stop=False)
                    nc.tensor.matmul(ps_a, lhsT=ed_b[:NB, 0:NA], rhs=ed_b[:NB],
                                     start=False, stop=True)

                def emit_mm2_b():
                    nc.tensor.matmul(ps_b[:NB], lhsT=ed_a[:, NA:N], rhs=ed_a,
                                     start=True, stop=False)
                    nc.tensor.matmul(ps_b[:NB], lhsT=ed_b[:NB, NA:N],
                                     rhs=ed_b[:NB], start=False, stop=True)

                # softmax over the free axis. exp(relu(y)) == max(exp(y), 1)
                # exactly, so skip the relu pass: ACT computes exp(scale*y)
                # straight from PSUM, DVE's 2x-mode tensor_scalar applies the
                # max and emits the row-sum for free via accum_out. No max-
                # subtraction needed: scores <= N/sqrt(C) ~ 15, exp stays
                # comfortably inside fp32 range.
                def emit_exp_a():
                    nc.scalar.activation(exp_a, ps_a, AFT.Exp, scale=scale)

                def emit_exp_b():
                    nc.scalar.activation(exp_b[:NB], ps_b[:NB], AFT.Exp,
                                         scale=scale)

                if i == BLOC - 1:
                    emit_mm2_b(); emit_mm2_a(); emit_exp_b(); emit_exp_a()
                else:
                    emit_mm2_a(); emit_mm2_b(); emit_exp_a(); emit_exp_b()

                # Software-pipeline the DVE softmax tail: batch i's tail is
                # emitted after batch i+1's reduce so the reduce stays ahead
                # of it in the DVE stream (otherwise the reduce chains behind
                # the whole previous batch and the cadence blows past the DMA
                # rate).
                if pending_tail is not None:
                    pending_tail(red_inst)
                pending_tail = make_tail(i, exp_a, exp_b)

            pending_tail(None)

            # Two HWDGE rings so the stores overlap; acc_b completes first
            # and goes on the sync ring (idle since the last x load).
            nc.sync.dma_start(out=out[NA:N, :], in_=acc_b[:NB])
            nc.scalar.dma_start(out=out[0:NA, :], in_=acc_a)

    nc.compile()
    return nc


_NC_CACHE = None


def _get_nc():
    global _NC_CACHE
    if _NC_CACHE is None:
        _NC_CACHE = _build_kernel()
    return _NC_CACHE


def kernel(x, E_s, _trace=False, _trace_kwargs=None):
    assert x.shape == (B, C, N, T) and E_s.shape == (C, N)
    x = np.ascontiguousarray(x, dtype=np.float32)
    E_s = np.ascontiguousarray(E_s, dtype=np.float32)
    xr = x.reshape(B, C, N * T)

    nc = _get_nc()
    in_maps = [
        {"x": xr[i * BLOC:(i + 1) * BLOC], "E_s": E_s} for i in range(NCORES)
    ]
    kwargs = {}
    if _trace:
        kwargs = dict(trace=True, **(_trace_kwargs or {}))
    res = run_bass_kernel_spmd(nc, in_maps, core_ids=list(range(NCORES)), **kwargs)

    total = np.zeros((N, N), dtype=np.float32)
    for r in res.results:
        total += r["acc"]
    a_mean = total / np.float32(B)
    out = (a_mean > 0.5).astype(np.float32)
    if _trace:
        return out, res
    return out


if __name__ == "__main__":
    rng = np.random.default_rng(0)
    x = rng.standard_normal((B, C, N, T), dtype=np.float32)
    E_s = (rng.random((C, N), dtype=np.float32) - 0.5) * 0.2
    print(kernel(x, E_s).sum())

